# revision 1
# baseline (speedup 1.0000x reference)
"""Trainium2 Bass kernel for DDGAttention (B=4, L=2048, D=256, H=8, DQK=DV=32).

Sharding: 8 cores = 4 batches x 2 query-halves; each core handles 1024 queries
against all 2048 keys of its batch. Inputs are rolled per core so local queries
are always rows 0..1023 (attention is key-order invariant, so rolling keys is
safe as long as k/v/pos_CB are rolled together).

Algebra: rel_pos (B,L,L,3) never materializes. Softmax rows sum to 1, so
  atom_pos_bias = alpha @ pos_CB - pos_CA.
V is augmented to [v | pos_CB | 1] (36 cols/head); a single PE accumulation
yields feat_node, alpha@pos_CB and the softmax denominator in one pass, and the
exp never needs a running-max (logits are O(3) by construction).
sqrt/rsqrt are computed as exp(+-0.5*ln(x)) so every ACT op lives in the single
natural_log_exp table set (no mid-kernel table reloads).
"""
import sys

sys.path.insert(0, "/opt/trn_rl_repo")

import numpy as np

B, L, DIN, DOUT = 4, 2048, 256, 256
H, DQK, DV = 8, 32, 32
NQ = L // 2          # queries per core
LB = L // 128        # 16 key/row blocks
NG = NQ // 512       # query groups of 512
QB = NQ // 128       # 8 query blocks per core

_cache = {}


def _build(phases=('pro', 'A', 'B')):
    import concourse.mybir as mybir
    import concourse.tile as tile
    from concourse import bacc, library_config
    from concourse.masks import make_identity

    F32 = mybir.dt.float32
    BF16 = mybir.dt.bfloat16
    F32R = mybir.dt.float32r
    AF = mybir.ActivationFunctionType
    ALU = mybir.AluOpType
    AX = mybir.AxisListType

    # Force exp/ln to resolve to the combined natural_log_exp set so the
    # act-table-load pass never thrashes between per-function home sets.
    import concourse.bacc as bacc_mod
    real_tables = bacc_mod.get_activation_tables("gen3")
    patched = {}
    for name, funcs in real_tables.items():
        funcs = set(funcs)
        if name != "natural_log_exp_and_others":
            funcs.discard(mybir.ActivationFunctionType.Exp)
            funcs.discard(mybir.ActivationFunctionType.Ln)
        patched[name] = funcs
    bacc_mod.get_activation_tables = lambda arch, _p=patched: _p

    nc = bacc.Bacc("TRN2", target_bir_lowering=False, debug=False, num_devices=8)

    x_d = nc.dram_tensor("x", [L, DIN], F32, kind="ExternalInput")
    pcb_d = nc.dram_tensor("pcb", [L, 3], F32, kind="ExternalInput")
    pca_d = nc.dram_tensor("pca", [NQ, 3], F32, kind="ExternalInput")
    frm_d = nc.dram_tensor("frm", [NQ, 9], F32, kind="ExternalInput")
    wq_d = nc.dram_tensor("wq", [DIN, 256], F32R, kind="ExternalInput")
    wk_d = nc.dram_tensor("wk", [DIN, 256], F32R, kind="ExternalInput")
    wv_d = nc.dram_tensor("wv", [DIN, 256], F32R, kind="ExternalInput")
    won_d = nc.dram_tensor("won", [32, 8 * 256], F32R, kind="ExternalInput")
    wos_d = nc.dram_tensor("wos", [57, 256], F32R, kind="ExternalInput")
    g_d = nc.dram_tensor("g", [1, 256], F32, kind="ExternalInput")
    bb_d = nc.dram_tensor("bb", [1, 256], F32, kind="ExternalInput")
    out_d = nc.dram_tensor("out", [NQ, DOUT], F32, kind="ExternalOutput")

    with tile.TileContext(nc) as tc, tc.tile_pool(name="per", bufs=1) as per:
        nc.gpsimd.load_library(library_config.attn)

        ident = per.tile([128, 128], F32)
        make_identity(nc, ident[:])

        # persistent SBUF tensors
        x_sb = per.tile([128, LB * 256], F32)    # x[128b+p, d] -> [p, b*256+d]
        xT_sb = per.tile([128, 2 * L], F32R)      # xT[p, c*L+l] = x[l, 128c+p]
        kT_sb = per.tile([128, 2 * L], F32R)      # kT[p, c*L+l] = k[l, 128c+p]
        qT_sb = per.tile([128, 2 * NQ], F32R)
        v2_sb = per.tile([128, LB * 288], BF16)   # key blk: (h,36) = [v|pcb|1]
        wq_sb = per.tile([128, 2 * 256], F32R)    # W[128c+p, o] -> [p, c*256+o]
        wk_sb = per.tile([128, 2 * 256], F32R)
        wv_sb = per.tile([128, 2 * 256], F32R)
        won_sb = per.tile([32, 8 * 256], F32R)    # [d, h*256+o] = Wo[32h+d, o]
        wos_sb = per.tile([57, 256], F32R)        # rows: 56 spatial + bias row
        gg_sb = per.tile([128, 256], F32)
        bb_sb = per.tile([128, 256], F32)
        fnT = [per.tile([32, NQ], F32R, name=f"fnT{h}") for h in range(H)]
        wpT = [per.tile([3, NQ], F32, name=f"wpT{h}") for h in range(H)]
        b30 = per.tile([128, 1], F32)
        b5 = per.tile([128, 1], F32)
        c256 = per.tile([128, 1], F32)
        c10 = per.tile([128, 1], F32)
        nc.vector.memset(b30[:], 1e-30)
        nc.vector.memset(b5[:], 1e-5)
        nc.vector.memset(c256[:], 1.0 / 256)
        nc.vector.memset(c10[:], 1e-10)
        ones32 = per.tile([1, 32], F32)
        nc.vector.memset(ones32[:], 1.0)

        # input loads
        for dk in range(8):
            nc.sync.dma_start(
                x_sb[:, dk * 512 : dk * 512 + 512].rearrange(
                    "p (b d) -> p b d", d=256
                ),
                x_d.rearrange("(b p) d -> p b d", p=128)[:, 2 * dk : 2 * dk + 2, :],
            )
        for w_sb, w_d in ((wq_sb, wq_d), (wk_sb, wk_d), (wv_sb, wv_d)):
            nc.sync.dma_start(
                w_sb[:].rearrange("p (c d) -> p c d", d=256),
                w_d.rearrange("(c p) d -> p c d", p=128),
            )
        nc.sync.dma_start(won_sb[:], won_d[:])
        nc.sync.dma_start(wos_sb[:], wos_d[:])
        g1 = per.tile([1, 256], F32)
        b1 = per.tile([1, 256], F32)
        nc.sync.dma_start(g1[:], g_d[:])
        nc.sync.dma_start(b1[:], bb_d[:])
        nc.gpsimd.partition_broadcast(gg_sb[:], g1[:])
        nc.gpsimd.partition_broadcast(bb_sb[:], b1[:])

        # ---------- prologue: xT, kT, qT, v/V2 ----------
        with tc.tile_pool(name="pro_ps", bufs=2, space="PSUM") as pro_ps, \
             tc.tile_pool(name="pro_sb", bufs=3) as pro_sb:
            for blk in range(LB):
                for c in range(2):
                    tps = pro_ps.tile([128, 128], F32, tag="tp")
                    nc.tensor.transpose(
                        tps[:],
                        x_sb[:, blk * 256 + c * 128 : blk * 256 + c * 128 + 128],
                        ident[:],
                    )
                    nc.vector.tensor_copy(
                        xT_sb[:, c * L + blk * 128 : c * L + blk * 128 + 128],
                        tps[:],
                    )

            # kT[hd, l] = Wk.T @ xT ; lhsT = Wk chunk [din128, hd128]
            for hc in range(2):
                for lg in range(L // 512):
                    kps = pro_ps.tile([128, 512], F32, tag="proj", bufs=4)
                    for dc in range(2):
                        nc.tensor.matmul(
                            kps[:],
                            wk_sb[:, dc * 256 + hc * 128 : dc * 256 + hc * 128 + 128],
                            xT_sb[:, dc * L + lg * 512 : dc * L + lg * 512 + 512],
                            start=(dc == 0),
                            stop=(dc == 1),
                        )
                    nc.scalar.copy(
                        kT_sb[:, hc * L + lg * 512 : hc * L + lg * 512 + 512],
                        kps[:],
                    )
            # qT (only local 1024 query columns)
            for hc in range(2):
                for qg in range(NQ // 512):
                    qps = pro_ps.tile([128, 512], F32, tag="proj", bufs=4)
                    for dc in range(2):
                        nc.tensor.matmul(
                            qps[:],
                            wq_sb[:, dc * 256 + hc * 128 : dc * 256 + hc * 128 + 128],
                            xT_sb[:, dc * L + qg * 512 : dc * L + qg * 512 + 512],
                            start=(dc == 0),
                            stop=(dc == 1),
                        )
                    nc.scalar.copy(
                        qT_sb[:, hc * NQ + qg * 512 : hc * NQ + qg * 512 + 512],
                        qps[:],
                    )
            # v rows + V2 assembly: v[l, hd] ; lhsT = xT chunk [din128, l128]
            for blk in range(LB):
                vps = pro_ps.tile([128, 256], F32, tag="vproj")
                for dc in range(2):
                    nc.tensor.matmul(
                        vps[:],
                        xT_sb[:, dc * L + blk * 128 : dc * L + blk * 128 + 128],
                        wv_sb[:, dc * 256 : dc * 256 + 256],
                        start=(dc == 0),
                        stop=(dc == 1),
                    )
                v2b = v2_sb[:, blk * 288 : blk * 288 + 288].rearrange(
                    "p (h c) -> p h c", c=36
                )
                nc.vector.tensor_copy(
                    v2b[:, :, 0:32], vps[:].rearrange("p (h d) -> p h d", d=32)
                )
                pcb_t = pro_sb.tile([128, 3], F32, tag="pcb")
                nc.sync.dma_start(pcb_t[:], pcb_d[blk * 128 : blk * 128 + 128, :])
                nc.vector.tensor_copy(
                    v2b[:, :, 32:35], pcb_t[:, None, :].broadcast_to([128, 8, 3])
                )
                nc.gpsimd.memset(v2b[:, :, 35:36], 1.0)

        # ---------- phase A: attention ----------
        with tc.tile_pool(name="st_ps", bufs=3, space="PSUM") as st_ps, \
             tc.tile_pool(name="av_ps", bufs=2, space="PSUM") as av_ps, \
             tc.tile_pool(name="exp_sb", bufs=6) as exp_sb, \
             tc.tile_pool(name="nrm_sb", bufs=3) as nrm_sb:
            for h in (range(H) if "A" in phases else []):
                hc, hr = h // 4, 32 * (h % 4)
                for g in range(NG):
                    av = av_ps.tile([36, 512], F32, tag="av")
                    sts = {}
                    for kp in range(LB // 2 + 1):
                        if kp < LB // 2:
                            st = st_ps.tile([128, 1024], F32, tag="st")
                            for i in range(2):
                                kc = 2 * kp + i
                                nc.tensor.matmul(
                                    st[:, i * 512 : i * 512 + 512],
                                    kT_sb[hr : hr + 32,
                                          hc * L + kc * 128 : hc * L + kc * 128 + 128],
                                    qT_sb[hr : hr + 32,
                                          hc * NQ + g * 512 : hc * NQ + g * 512 + 512],
                                    start=True,
                                    stop=True,
                                    tile_position=(hr, 0),
                                )
                            sts[kp] = st
                        if kp >= 1:
                            kpp = kp - 1
                            ex = exp_sb.tile([128, 1024], BF16, tag="ex")
                            nc.scalar.activation(ex[:], sts.pop(kpp)[:], AF.Exp)
                            for i in range(2):
                                kc = 2 * kpp + i
                                nc.tensor.matmul(
                                    av[:],
                                    v2_sb[:, kc * 288 : kc * 288 + 288].rearrange(
                                        "p (h c) -> p h c", c=36
                                    )[:, h, :],
                                    ex[:, i * 512 : i * 512 + 512],
                                    start=(kc == 0),
                                    stop=(kc == LB - 1),
                                )
                    # normalization: r = 1/sumexp, broadcast, apply
                    pp = nrm_sb.tile([36, 512], F32, tag="pp")
                    nc.vector.tensor_copy(pp[32:36, :], av[32:36, :])
                    ps_pos = nrm_sb.tile([3, 512], F32, tag="pspos")
                    se = nrm_sb.tile([1, 512], F32, tag="se")
                    nc.sync.dma_start(ps_pos[:], pp[32:35, :])
                    nc.sync.dma_start(se[:], pp[35:36, :])
                    r = nrm_sb.tile([1, 512], F32, tag="r")
                    nc.vector.reciprocal(r[:], se[:])
                    rb = nrm_sb.tile([32, 512], F32, tag="rb")
                    nc.gpsimd.partition_broadcast(rb[:], r[:])
                    nc.vector.tensor_tensor(
                        fnT[h][:, g * 512 : g * 512 + 512], av[0:32, :], rb[:],
                        op=ALU.mult,
                    )
                    nc.vector.tensor_tensor(
                        wpT[h][:, g * 512 : g * 512 + 512], ps_pos[:], rb[0:3, :],
                        op=ALU.mult,
                    )

        # ---------- phase B: spatial features + out proj + LN ----------
        with tc.tile_pool(name="tp_ps", bufs=3, space="PSUM") as tp_ps, \
             tc.tile_pool(name="op_ps", bufs=2, space="PSUM") as op_ps, \
             tc.tile_pool(name="bp", bufs=3) as bp:
            for qb in (range(QB) if "B" in phases else []):
                q0 = qb * 128
                # transpose wposT [3,128] x8 -> wq_t [128, 24]
                wq_t = tp_ps.tile([128, 24], F32, tag="wq")
                for h in range(H):
                    nc.tensor.transpose(
                        wq_t[:, 3 * h : 3 * h + 3],
                        wpT[h][:, q0 : q0 + 128],
                        ident[0:3, 0:3],
                    )
                pca_t = bp.tile([128, 3], F32, tag="pca")
                nc.sync.dma_start(pca_t[:], pca_d[q0 : q0 + 128, :])
                frm_t = bp.tile([128, 9], F32, tag="frm")
                nc.sync.dma_start(frm_t[:], frm_d[q0 : q0 + 128, :])

                apb = bp.tile([128, 24], F32, tag="apb")
                apb3 = apb.rearrange("p (h j) -> p h j", j=3)
                nc.vector.tensor_tensor(
                    apb3, wq_t[:].rearrange("p (h j) -> p h j", j=3),
                    pca_t[:, None, :].broadcast_to([128, 8, 3]), op=ALU.subtract,
                )
                fsp = bp.tile([128, 57], F32, tag="fsp")
                fsp_p = fsp[:, 0:24].rearrange("p (h i) -> p h i", i=3)
                tmp = bp.tile([128, 24], F32, tag="tmp")
                tmp3 = tmp.rearrange("p (h j) -> p h j", j=3)
                for i in range(3):
                    nc.vector.tensor_tensor(
                        tmp3, apb3,
                        frm_t[:, 3 * i : 3 * i + 3][:, None, :].broadcast_to(
                            [128, 8, 3]
                        ),
                        op=ALU.mult,
                    )
                    nc.vector.tensor_reduce(
                        fsp_p[:, :, i], tmp3, axis=AX.X, op=ALU.add
                    )
                # distance = sqrt(sum apb^2) = exp(.5 ln)
                sq = bp.tile([128, 24], F32, tag="sq")
                nc.vector.tensor_tensor(sq[:], apb[:], apb[:], op=ALU.mult)
                d2 = bp.tile([128, 8], F32, tag="d2")
                nc.vector.tensor_reduce(
                    d2[:], sq.rearrange("p (h j) -> p h j", j=3), axis=AX.X,
                    op=ALU.add,
                )
                nc.scalar.activation(d2[:], d2[:], AF.Ln, bias=b30[:])
                nc.scalar.activation(fsp[:, 24:32], d2[:], AF.Exp, scale=0.5)
                # direction = fp / (|fp| + 1e-10)
                nc.vector.tensor_tensor(sq[:], fsp[:, 0:24], fsp[:, 0:24],
                                        op=ALU.mult)
                f2 = bp.tile([128, 8], F32, tag="f2")
                nc.vector.tensor_reduce(
                    f2[:], sq.rearrange("p (h j) -> p h j", j=3), axis=AX.X,
                    op=ALU.add,
                )
                nc.scalar.activation(f2[:], f2[:], AF.Ln, bias=b30[:])
                nc.scalar.activation(f2[:], f2[:], AF.Exp, scale=0.5)
                nc.vector.tensor_tensor(
                    f2[:], f2[:], c10[:].broadcast_to([128, 8]), op=ALU.add
                )
                nc.vector.reciprocal(f2[:], f2[:])
                nc.vector.tensor_tensor(
                    fsp[:, 32:56].rearrange("p (h i) -> p h i", i=3), fsp_p,
                    f2[:, :, None].broadcast_to([128, 8, 3]), op=ALU.mult,
                )
                nc.vector.memset(fsp[:, 56:57], 1.0)
                # transpose spatial features -> [57, 128]
                fspT_ps = tp_ps.tile([57, 128], F32, tag="fspT")
                nc.tensor.transpose(fspT_ps[:], fsp[:], ident[:])
                fspT = bp.tile([57, 128], F32R, tag="fspTs")
                nc.scalar.copy(fspT[:], fspT_ps[:])
                # output projection (+bias via ones row)
                o_t = op_ps.tile([128, 256], F32, tag="o")
                for h in range(H):
                    nc.tensor.matmul(
                        o_t[:],
                        fnT[h][:, q0 : q0 + 128],
                        won_sb[:, h * 256 : h * 256 + 256],
                        start=(h == 0),
                        stop=False,
                    )
                nc.tensor.matmul(o_t[:], fspT[:], wos_sb[:], start=False, stop=True)
                # residual + layernorm (validated primitive ops only)
                y = bp.tile([128, 256], F32, tag="y")
                nc.vector.tensor_tensor(
                    y[:], o_t[:], x_sb[:, qb * 256 : qb * 256 + 256], op=ALU.add
                )
                m = bp.tile([128, 1], F32, tag="m")
                nc.vector.tensor_reduce(m[:], y[:], axis=AX.X, op=ALU.add)
                nc.vector.tensor_tensor(m[:], m[:], c256[:], op=ALU.mult)
                cent = bp.tile([128, 256], F32, tag="cent")
                nc.vector.tensor_tensor(
                    cent[:], y[:], m[:].broadcast_to([128, 256]), op=ALU.subtract
                )
                sqs = bp.tile([128, 256], F32, tag="sqs")
                nc.vector.tensor_tensor(sqs[:], cent[:], cent[:], op=ALU.mult)
                var = bp.tile([128, 1], F32, tag="var")
                nc.vector.tensor_reduce(var[:], sqs[:], axis=AX.X, op=ALU.add)
                nc.scalar.activation(var[:], var[:], AF.Ln, bias=b5[:],
                                     scale=1.0 / 256)
                nc.scalar.activation(var[:], var[:], AF.Exp, scale=-0.5)
                ob = bp.tile([128, 256], F32, tag="ob")
                nc.vector.tensor_tensor(
                    ob[:], cent[:], var[:].broadcast_to([128, 256]), op=ALU.mult
                )
                nc.vector.tensor_tensor(ob[:], ob[:], gg_sb[:], op=ALU.mult)
                nc.vector.tensor_tensor(ob[:], ob[:], bb_sb[:], op=ALU.add)
                nc.sync.dma_start(out_d[q0 : q0 + 128, :], ob[:])

    nc.compile()
    return nc


def _prep_inputs(x, pos_CA, pos_CB, frame, Wq, Wk, Wv, Wo, bo, ln_g, ln_b):
    won = (
        np.ascontiguousarray(Wo[:256].reshape(8, 32, 256).transpose(1, 0, 2))
        .reshape(32, 8 * 256)
        .astype(np.float32)
    )
    wos = np.concatenate([Wo[256:312], bo[None, :]], axis=0).astype(np.float32)
    maps = []
    for core in range(8):
        b, qh = core // 2, core % 2
        roll = -qh * NQ
        maps.append(
            {
                "x": np.ascontiguousarray(np.roll(x[b], roll, axis=0)),
                "pcb": np.ascontiguousarray(np.roll(pos_CB[b], roll, axis=0)),
                "pca": np.ascontiguousarray(pos_CA[b, qh * NQ : (qh + 1) * NQ]),
                "frm": np.ascontiguousarray(
                    frame[b, qh * NQ : (qh + 1) * NQ].reshape(NQ, 9)
                ),
                "wq": Wq,
                "wk": Wk,
                "wv": Wv,
                "won": won,
                "wos": wos,
                "g": ln_g[None, :],
                "bb": ln_b[None, :],
            }
        )
    return maps


def kernel(**inputs):
    from concourse.bass_utils import run_bass_kernel_spmd

    inputs = {k: np.asarray(v, dtype=np.float32) for k, v in inputs.items()}
    if "nc" not in _cache:
        _cache["nc"] = _build()
    nc = _cache["nc"]
    in_maps = _prep_inputs(**inputs)
    res = run_bass_kernel_spmd(nc, in_maps, list(range(8)))
    out = np.empty((B, L, DOUT), dtype=np.float32)
    for core in range(8):
        b, qh = core // 2, core % 2
        out[b, qh * NQ : (qh + 1) * NQ] = res.results[core]["out"]
    return out

